# revision 3
# baseline (speedup 1.0000x reference)
"""Trainium2 Bass kernel for nn_Attention_35905926595471.

Channel-attention (XCA-style) block, data-parallel over batch: 8 samples on
8 NeuronCores. Per core: FiLM folded into per-sample qkv weights on the host
(bias enters via a ones-channel in the contraction), qkv 1x1 conv on PE (bf16),
3x3 depthwise conv split across engines: v-tiles as diagonal-weight matmul taps
accumulating in PSUM, q/k-tiles as 4x tensor-scalar muls (DVE/ACT/Pool) plus
adds on DVE and compute-capable DMA (accum_op=add). Staged planes hold two
16-row chunks with zero-padded 130-col rows; per-head Grams come from
DMA-transposed conv outputs, norms from the Gram diagonal, softmax smalls, and
the attention map folded into the output projection before the final matmul.
"""
import numpy as np
from contextlib import ExitStack

import concourse.bacc as bacc
import concourse.bass as bass
import concourse.mybir as mybir
from concourse import tile
from concourse.bass_utils import run_bass_kernel_spmd

F32 = mybir.dt.float32
BF16 = mybir.dt.bfloat16
NPBF16 = mybir.dt.np(BF16)
AL = mybir.AluOpType
AX = mybir.AxisListType
AF = mybir.ActivationFunctionType

DIM, HEADS, H, W = 192, 6, 128, 128
HD = DIM // HEADS          # 32
N = H * W                  # 16384
CH = 2048                  # px per chunk (16 rows)
SW = W + 2                 # padded row stride 130
QROWS = 34                 # quarter plane rows: 32 image rows + 2 halo
PROWS = 18                 # ot4 pair plane rows: 16 image rows + 2 halo


def _perm():
    perm = []
    for t in range(3):
        for h in (2 * t, 2 * t + 1):
            perm += list(range(h * HD, (h + 1) * HD))
            perm += list(range(DIM + h * HD, DIM + (h + 1) * HD))
    perm += list(range(2 * DIM, 3 * DIM))
    return np.array(perm)


def _emit(nc, t):
    with ExitStack() as ctx:
        tc = ctx.enter_context(tile.TileContext(nc))
        sb = ctx.enter_context(tc.tile_pool(name="sb", bufs=1))
        plp = ctx.enter_context(tc.tile_pool(name="planes", bufs=2))
        acp = ctx.enter_context(tc.tile_pool(name="accq", bufs=2))
        qtp = ctx.enter_context(tc.tile_pool(name="qkt", bufs=2))
        scr = ctx.enter_context(tc.tile_pool(name="scr", bufs=1))
        ysp = ctx.enter_context(tc.tile_pool(name="ys", bufs=2))
        pq = ctx.enter_context(tc.tile_pool(name="pq", bufs=2, space=bass.MemorySpace.PSUM))
        pb = ctx.enter_context(tc.tile_pool(name="pb", bufs=2, space=bass.MemorySpace.PSUM))

        # ---- resident tensors ----
        xs = sb.tile([128, 2 * N], BF16, tag="xs", name="xs")
        vsb = sb.tile([128, N + N // 2], BF16, tag="vsb", name="vsb")
        wq1 = sb.tile([128, 640], BF16, tag="wq1", name="wq1")
        wq2 = sb.tile([65, 640], BF16, tag="wq2", name="wq2")
        wdw = sb.tile([128, 45], F32, tag="wdw", name="wdw")
        dwd = sb.tile([128, 18 * 128], BF16, tag="dwd", name="dwd")
        wpT = [sb.tile([128, DIM], F32, tag="wpT0", name="wpT0"),
               sb.tile([64, DIM], F32, tag="wpT1", name="wpT1")]
        idf = sb.tile([128, 128], F32, tag="idf", name="idf")
        tmpc = sb.tile([128, 3], F32, tag="tmpc", name="tmpc")
        onesr = sb.tile([1, 128], F32, tag="onesr", name="onesr")
        sm = sb.tile([128, 16], F32, tag="sm", name="sm")
        Lsb = [sb.tile([128, 128], F32, tag=f"L{g}", name=f"L{g}") for g in range(3)]
        dscr = sb.tile([128, 128], F32, tag="dscr", name="dscr")
        nrow = [sb.tile([1, 128], F32, tag=f"nrow{g}", name=f"nrow{g}") for g in range(3)]
        Asb = [sb.tile([128, DIM], F32, tag="A0", name="A0"), sb.tile([64, DIM], F32, tag="A1", name="A1")]
        w2t1 = sb.tile([128, DIM], BF16, tag="w2t1", name="w2t1")
        w2t2 = sb.tile([128, DIM], BF16, tag="w2t2", name="w2t2")

        nc.sync.dma_start(wq1[:], t["wq1"].ap()[:, :])
        nc.sync.dma_start(wq2[:], t["wq2"].ap()[:, :])
        nc.sync.dma_start(wdw[:], t["wdw"].ap()[:, :])
        nc.sync.dma_start(dwd[:], t["dwd"].ap()[:, :])
        nc.sync.dma_start(wpT[0][:], t["wpT"].ap()[0:128, :])
        nc.sync.dma_start(wpT[1][:], t["wpT"].ap()[128:192, :])
        nc.sync.dma_start(idf[:], t["idf"].ap()[:, :])
        nc.sync.dma_start(tmpc[:], t["tmpc"].ap()[:, :])
        nc.sync.dma_start(onesr[:], t["onesr"].ap()[:, :])
        for ci in range(8):
            nc.sync.dma_start(xs[:, ci * CH:(ci + 1) * CH], t["xa"].ap()[:, ci * CH:(ci + 1) * CH])
            nc.sync.dma_start(xs[0:65, N + ci * CH:N + (ci + 1) * CH], t["xb"].ap()[:, ci * CH:(ci + 1) * CH])

        def wcol(g, tap):
            return wdw[0:128, g * 9 + tap:g * 9 + tap + 1]

        def qkv_psum(g, ci, k):
            """qkv matmul for 1024 px -> [128,1024] psum (full-partition tiles)."""
            ps = pq.tile([128, 1024], F32, tag="mm", name="mm")
            px = ci * CH + k * 1024
            for q in range(2):
                nc.tensor.matmul(ps[:, q * 512:(q + 1) * 512], wq1[:, g * 128:(g + 1) * 128],
                                 xs[:, px + q * 512:px + q * 512 + 512], start=True, stop=False)
                nc.tensor.matmul(ps[:, q * 512:(q + 1) * 512], wq2[0:65, g * 128:(g + 1) * 128],
                                 xs[0:65, N + px + q * 512:N + px + q * 512 + 512], start=False, stop=True)
            return ps

        # ================= q/k tiles (g = 0,1,2) and v ot3 (g = 3) =================
        def emit_planar_ot(g, taps_fn):
            quarters = [None] * 4

            def newq(qi):
                quarters[qi] = plp.tile([128, QROWS * SW], BF16, tag="pl", name="pl")
                z3 = quarters[qi][:].rearrange("p (r c) -> p r c", c=SW)
                nc.gpsimd.memset(z3[:, :, 0:1], 0.0)
                nc.gpsimd.memset(z3[:, :, 129:130], 0.0)
                if qi == 0:
                    nc.gpsimd.memset(z3[:, 0:1, :], 0.0)
                if qi == 3:
                    nc.gpsimd.memset(z3[:, 33:34, :], 0.0)

            def evict(ci, k, extra):
                ps = qkv_psum(g, ci, k)
                p3 = ps[:].rearrange("p (r c) -> p r c", c=W)
                qi = ci // 2
                s3 = quarters[qi][:].rearrange("p (r c) -> p r c", c=SW)
                r0 = 1 + 16 * (ci % 2) + 8 * k
                nc.scalar.activation(s3[:, r0:r0 + 8, 1:129], p3[:], AF.Identity, scale=1.0)
                if extra == "prev":
                    sp = quarters[qi - 1][:].rearrange("p (r c) -> p r c", c=SW)
                    nc.scalar.activation(sp[:, 33:34, 1:129], p3[:, 0:1, :], AF.Identity, scale=1.0)
                if extra == "next":
                    sn = quarters[qi + 1][:].rearrange("p (r c) -> p r c", c=SW)
                    nc.scalar.activation(sn[:, 0:1, 1:129], p3[:, 7:8, :], AF.Identity, scale=1.0)

            newq(0)
            for qi in range(4):
                evict(2 * qi, 0, "prev" if qi > 0 else None)
                evict(2 * qi, 1, None)
                if qi > 0:
                    taps_fn(g, 2 * qi - 1, quarters[qi - 1])
                evict(2 * qi + 1, 0, None)
                taps_fn(g, 2 * qi, quarters[qi])
                if qi < 3:
                    newq(qi + 1)
                evict(2 * qi + 1, 1, "next" if qi < 3 else None)
            taps_fn(g, 7, quarters[3])

        # ---- q/k taps: DVE muls/adds + ACT mul + Pool muls + DMA CCE adds ----
        def qk_taps(g, ci, quart):
            lb = 16 * (ci % 2)
            s3 = quart[:].rearrange("p (r c) -> p r c", c=SW)

            def view(tap):
                dy, dx = tap // 3, tap % 3
                return s3[:, lb + dy:lb + dy + 16, dx:dx + 128]

            def v3(tl):
                return tl[:].rearrange("p (r c) -> p r c", c=W)

            acc = acp.tile([128, CH], BF16, tag="acc", name="acc")
            nc.vector.tensor_scalar_mul(v3(acc), view(4), wcol(g, 4))
            pA = scr.tile([128, CH], BF16, tag="sA", name="sA")
            pB = scr.tile([128, CH], BF16, tag="sB", name="sB")
            pC = scr.tile([128, CH], BF16, tag="sC", name="sC")
            pD = scr.tile([128, CH], BF16, tag="sD", name="sD")
            pE = scr.tile([128, CH], BF16, tag="sE", name="sE")
            nc.scalar.mul(v3(pA), view(0), wcol(g, 0))                # ACT
            nc.gpsimd.tensor_scalar_mul(v3(pB), view(2), wcol(g, 2))  # Pool
            nc.gpsimd.tensor_scalar_mul(v3(pC), view(6), wcol(g, 6))  # Pool
            nc.vector.tensor_scalar_mul(v3(pD), view(1), wcol(g, 1))  # DVE
            nc.vector.tensor_scalar_mul(v3(pE), view(3), wcol(g, 3))  # DVE
            nc.gpsimd.dma_start(pA[:], pB[:], accum_op=AL.add)        # CCE: t0+t2
            nc.gpsimd.dma_start(pD[:], pE[:], accum_op=AL.add)        # CCE: t1+t3
            pB2 = scr.tile([128, CH], BF16, tag="sB", name="sB")
            pE2 = scr.tile([128, CH], BF16, tag="sE", name="sE")
            nc.vector.tensor_scalar_mul(v3(pB2), view(5), wcol(g, 5))  # DVE
            nc.vector.tensor_scalar_mul(v3(pE2), view(7), wcol(g, 7))  # DVE
            nc.vector.tensor_add(pB2[:], pB2[:], pE2[:])              # t5+t7
            pE3 = scr.tile([128, CH], BF16, tag="sE", name="sE")
            nc.vector.tensor_scalar_mul(v3(pE3), view(8), wcol(g, 8))  # DVE
            nc.gpsimd.dma_start(pC[:], pE3[:], accum_op=AL.add)       # CCE: t6+t8
            nc.vector.tensor_add(acc[:], acc[:], pA[:])
            nc.vector.tensor_add(acc[:], acc[:], pD[:])
            nc.vector.tensor_add(acc[:], acc[:], pB2[:])
            nc.vector.tensor_add(acc[:], acc[:], pC[:])

            qt = qtp.tile([128, CH], BF16, tag="qt", name="qt")
            qt3 = qt[:].rearrange("p (b j) -> p b j", j=128)
            nc.sync.dma_start_transpose(qt3, acc[:])
            gram = grams[g]
            for b in range(16):
                nc.tensor.matmul(gram[:, 0:128], qt[:, b * 128:(b + 1) * 128],
                                 qt[:, b * 128:(b + 1) * 128],
                                 start=(ci == 0 and b == 0), stop=(ci == 7 and b == 15))

        # ---- v ot3 taps: 9 diagonal-weight matmuls per 1024-px PSUM ----
        def v3_taps(g, ci, quart):
            lb = 16 * (ci % 2)
            s3 = quart[:].rearrange("p (r c) -> p r c", c=SW)
            for k in range(2):
                tp = pb.tile([128, 1024], F32, tag="vt", name="vt")
                t3_ = tp[:].rearrange("p (r c) -> p r c", c=W)
                for q in range(2):
                    for tap in range(9):
                        dy, dx = tap // 3, tap % 3
                        r = lb + 8 * k + 4 * q + dy
                        nc.tensor.matmul(t3_[:, 4 * q:4 * q + 4, :], dwd[:, tap * 128:(tap + 1) * 128],
                                         s3[:, r:r + 4, dx:dx + 128], start=(tap == 0), stop=(tap == 8))
                nc.scalar.activation(vsb[0:128, ci * CH + k * 1024:ci * CH + k * 1024 + 1024],
                                     tp[:], AF.Identity, scale=1.0)

        # ================= qk pass + norms =================
        grams = {}
        for g in range(3):
            gt = pb.tile([128, 1024], F32, tag="vt", name=f"gram{g}")
            grams[g] = gt
            emit_planar_ot(g, qk_taps)
            # ---- norms + logits scale + softmax for tile g ----
            L = Lsb[g]
            gram = grams[g]
            nc.scalar.copy(L[:], gram[:, 0:128])
            dcol = sm[:, 9:10]
            scrc = sm[:, 10:11]
            dsc = sm[:, 11:12]
            nc.vector.tensor_mul(dscr[:], L[:], idf[:])
            nc.vector.reduce_sum(dcol, dscr[:], axis=AX.X)
            nc.scalar.sqrt(scrc, dcol)
            nc.vector.tensor_scalar_max(scrc, scrc, 1e-12)
            nc.vector.reciprocal(dsc, scrc)
            rs = sm[:, 12:13]
            nc.vector.tensor_mul(rs, dsc, tmpc[:, g:g + 1])
            pt = pq.tile([128, 1024], F32, tag="mm", name="pt")
            nc.tensor.transpose(pt[0:1, 0:128], dsc, idf[:])
            nc.scalar.copy(nrow[g][:], pt[0:1, 0:128])
            pt2 = pq.tile([128, 1024], F32, tag="mm", name="pt2")
            nc.tensor.matmul(pt2[:, 0:128], onesr[:], nrow[g][:], start=True, stop=True)
            nc.vector.tensor_scalar_mul(L[:], L[:], rs)
            nc.vector.tensor_mul(L[:], L[:], pt2[:, 0:128])
            for j in range(2):
                P0, K0 = 64 * j, 64 * j + 32
                mx = sm[P0:P0 + 32, 14:15]
                nc.vector.reduce_max(mx, L[P0:P0 + 32, K0:K0 + 32], axis=AX.X)
                nc.vector.tensor_scalar_sub(L[P0:P0 + 32, K0:K0 + 32], L[P0:P0 + 32, K0:K0 + 32], mx)
                nc.scalar.activation(L[P0:P0 + 32, K0:K0 + 32], L[P0:P0 + 32, K0:K0 + 32], AF.Exp)
                nc.vector.reduce_sum(mx, L[P0:P0 + 32, K0:K0 + 32], axis=AX.X)
                nc.vector.reciprocal(mx, mx)
                nc.vector.tensor_scalar_mul(L[P0:P0 + 32, K0:K0 + 32], L[P0:P0 + 32, K0:K0 + 32], mx)

        # ================= A_bd + W2T =================
        nc.gpsimd.memset(Asb[0][:], 0.0)
        nc.gpsimd.memset(Asb[1][:], 0.0)
        for h in range(HEADS):
            g, j = h // 2, h % 2
            src = Lsb[g][64 * j:64 * j + 32, 64 * j + 32:64 * j + 64]
            dst_t = Asb[0] if h < 4 else Asb[1]
            dp = 32 * (h % 4)
            dst = dst_t[dp:dp + 32, 32 * h:32 * h + 32]
            if dp == 64 * j:
                nc.vector.tensor_copy(dst, src)
            else:
                nc.sync.dma_start(dst, src)
        for dt_ in range(2):
            c0, cn = dt_ * 128, (128 if dt_ == 0 else 64)
            ps = pq.tile([128, 1024], F32, tag="mm", name="w2")
            nc.tensor.matmul(ps[0:cn, 0:DIM], Asb[0][:, c0:c0 + cn], wpT[0][:], start=True, stop=False)
            nc.tensor.matmul(ps[0:cn, 0:DIM], Asb[1][:, c0:c0 + cn], wpT[1][:], start=False, stop=True)
            if dt_ == 0:
                nc.scalar.copy(w2t1[:], ps[0:128, 0:DIM])
            else:
                nc.scalar.copy(w2t2[0:64, :], ps[0:64, 0:DIM])
        nc.sync.dma_start(w2t2[64:128, :], w2t2[0:64, :])

        # ================= v ot3 =================
        emit_planar_ot(3, v3_taps)

        # ================= v ot4 (64 chans, pairs packed on partition halves) ====
        pairs = [None] * 4

        def newpair(p):
            pairs[p] = plp.tile([128, PROWS * SW], BF16, tag="pp", name="pp")
            z3 = pairs[p][:].rearrange("p (r c) -> p r c", c=SW)
            nc.gpsimd.memset(z3[:, :, 0:1], 0.0)
            nc.gpsimd.memset(z3[:, :, 129:130], 0.0)
            if p == 0:
                nc.gpsimd.memset(z3[0:64, 0:1, :], 0.0)
            if p == 3:
                nc.gpsimd.memset(z3[64:128, 17:18, :], 0.0)

        def pair_evict(p, k):
            ps = pq.tile([128, 1024], F32, tag="mm", name="mm4")
            pxe = (2 * p) * CH + k * 1024
            pxo = (2 * p + 1) * CH + k * 1024
            for q in range(2):
                nc.tensor.matmul(ps[0:64, q * 512:(q + 1) * 512], wq1[:, 512:576],
                                 xs[:, pxe + q * 512:pxe + q * 512 + 512], start=True, stop=False)
                nc.tensor.matmul(ps[0:64, q * 512:(q + 1) * 512], wq2[0:65, 512:576],
                                 xs[0:65, N + pxe + q * 512:N + pxe + q * 512 + 512], start=False, stop=True)
                nc.tensor.matmul(ps[64:128, q * 512:(q + 1) * 512], wq1[:, 512:576],
                                 xs[:, pxo + q * 512:pxo + q * 512 + 512], start=True, stop=False)
                nc.tensor.matmul(ps[64:128, q * 512:(q + 1) * 512], wq2[0:65, 512:576],
                                 xs[0:65, N + pxo + q * 512:N + pxo + q * 512 + 512], start=False, stop=True)
            p3 = ps[:].rearrange("p (r c) -> p r c", c=W)
            s3 = pairs[p][:].rearrange("p (r c) -> p r c", c=SW)
            nc.scalar.activation(s3[:, 1 + 8 * k:9 + 8 * k, 1:129], p3[:], AF.Identity, scale=1.0)

        def pair_taps(p):
            s3 = pairs[p][:].rearrange("p (r c) -> p r c", c=SW)
            for k in range(2):
                tp = pb.tile([128, 1024], F32, tag="vt", name="vt4")
                t3_ = tp[:].rearrange("p (r c) -> p r c", c=W)
                for q in range(2):
                    for tap in range(9):
                        dy, dx = tap // 3, tap % 3
                        r = 8 * k + 4 * q + dy
                        nc.tensor.matmul(t3_[:, 4 * q:4 * q + 4, :], dwd[:, (9 + tap) * 128:(10 + tap) * 128],
                                         s3[:, r:r + 4, dx:dx + 128], start=(tap == 0), stop=(tap == 8))
                vc = N + 2048 * p + k * 1024
                nc.scalar.activation(vsb[0:64, vc:vc + 1024], tp[0:64, :], AF.Identity, scale=1.0)
                nc.scalar.activation(vsb[64:128, vc:vc + 1024], tp[64:128, :], AF.Identity, scale=1.0)

        def emit_y(ci):
            px = 1024 * ci
            c4 = ci // 2
            b2 = 64 * (c4 % 2)
            vcol = N + 2048 * (c4 // 2) + 1024 * (ci % 2)
            for oT, (o0, on) in enumerate([(0, 128), (128, 64)]):
                yp = pb.tile([128, 1024], F32, tag="vt", name="yp")
                for q in range(2):
                    nc.tensor.matmul(yp[0:on, q * 512:(q + 1) * 512], w2t1[:, o0:o0 + on],
                                     vsb[0:128, px + q * 512:px + q * 512 + 512], start=True, stop=False)
                    nc.tensor.matmul(yp[0:on, q * 512:(q + 1) * 512], w2t2[b2:b2 + 64, o0:o0 + on],
                                     vsb[b2:b2 + 64, vcol + q * 512:vcol + q * 512 + 512], start=False, stop=True)
                ys = ysp.tile([128, 1024], F32, tag=("ysA" if oT == 0 else "ysB"), name="ys")
                nc.scalar.copy(ys[0:on, :], yp[0:on, :])
                dst = t["yA"] if oT == 0 else t["yB"]
                nc.sync.dma_start(dst.ap()[:, px:px + 1024], ys[0:on, :])

        newpair(0)
        for p in range(4):
            pair_evict(p, 0)
            if p > 0:
                prev3 = pairs[p - 1][:].rearrange("p (r c) -> p r c", c=SW)
                cur3 = pairs[p][:].rearrange("p (r c) -> p r c", c=SW)
                nc.sync.dma_start(cur3[0:64, 0:1, 1:129], prev3[64:128, 16:17, 1:129])
            pair_evict(p, 1)
            cur3 = pairs[p][:].rearrange("p (r c) -> p r c", c=SW)
            nc.sync.dma_start(cur3[64:128, 0:1, 1:129], cur3[0:64, 16:17, 1:129])
            nc.sync.dma_start(cur3[0:64, 17:18, 1:129], cur3[64:128, 1:2, 1:129])
            if p > 0:
                prev3 = pairs[p - 1][:].rearrange("p (r c) -> p r c", c=SW)
                nc.sync.dma_start(prev3[64:128, 17:18, 1:129], cur3[0:64, 1:2, 1:129])
                pair_taps(p - 1)
                for ci in range(4 * (p - 1), 4 * p):
                    emit_y(ci)
            if p < 3:
                newpair(p + 1)
        pair_taps(3)
        for ci in range(12, 16):
            emit_y(ci)


_CACHE = {}


def _module():
    if "nc" in _CACHE:
        return _CACHE["nc"], _CACHE["t"]
    nc = bacc.Bacc("TRN2", target_bir_lowering=False, debug=False)
    t = {
        "xa": nc.dram_tensor("xa", [128, N], BF16, kind="ExternalInput"),
        "xb": nc.dram_tensor("xb", [65, N], BF16, kind="ExternalInput"),
        "wq1": nc.dram_tensor("wq1", [128, 640], BF16, kind="ExternalInput"),
        "wq2": nc.dram_tensor("wq2", [65, 640], BF16, kind="ExternalInput"),
        "wdw": nc.dram_tensor("wdw", [128, 45], F32, kind="ExternalInput"),
        "dwd": nc.dram_tensor("dwd", [128, 18 * 128], BF16, kind="ExternalInput"),
        "wpT": nc.dram_tensor("wpT", [192, DIM], F32, kind="ExternalInput"),
        "idf": nc.dram_tensor("idf", [128, 128], F32, kind="ExternalInput"),
        "tmpc": nc.dram_tensor("tmpc", [128, 3], F32, kind="ExternalInput"),
        "onesr": nc.dram_tensor("onesr", [1, 128], F32, kind="ExternalInput"),
        "yA": nc.dram_tensor("yA", [128, N], F32, kind="ExternalOutput"),
        "yB": nc.dram_tensor("yB", [64, N], F32, kind="ExternalOutput"),
    }
    _emit(nc, t)
    nc.compile()
    _CACHE["nc"], _CACHE["t"] = nc, t
    return nc, t


def kernel(x, k_v, w_kernel, w_qkv, w_dw, w_proj, temperature):
    x = np.asarray(x, np.float32)
    k_v = np.asarray(k_v, np.float32)
    w_kernel = np.asarray(w_kernel, np.float32)
    w_qkv = np.asarray(w_qkv, np.float32)
    w_dw = np.asarray(w_dw, np.float32)
    w_proj = np.asarray(w_proj, np.float32)
    temperature = np.asarray(temperature, np.float32).reshape(-1)

    perm = _perm()
    Wp = w_qkv[perm]                               # [576, 192]
    wdw_p = w_dw.reshape(3 * DIM, 9)[perm]         # [576, 9]
    # per-ot tap weight columns (f32) for tensor-scalar muls
    wdw_t = np.zeros((128, 45), np.float32)
    for g in range(4):
        wdw_t[:, g * 9:(g + 1) * 9] = wdw_p[g * 128:(g + 1) * 128]
    wdw_t[0:64, 36:45] = wdw_p[512:576]
    wdw_t[64:128, 36:45] = wdw_p[512:576]
    # diagonal matrices for PE conv taps: ot3 (9) then ot4 (9, repeated 64-diag)
    dwd = np.zeros((128, 18 * 128), np.float32)
    for tap in range(9):
        dwd[:, tap * 128:(tap + 1) * 128][np.arange(128), np.arange(128)] = wdw_p[384 + np.arange(128), tap]
        blk = dwd[:, (9 + tap) * 128:(10 + tap) * 128]
        blk[np.arange(64), np.arange(64)] = wdw_p[512 + np.arange(64), tap]
        blk[64 + np.arange(64), 64 + np.arange(64)] = wdw_p[512 + np.arange(64), tap]
    dwd = dwd.astype(NPBF16)

    wpT = np.ascontiguousarray(w_proj.T)
    idf = np.eye(128, dtype=np.float32)
    tmpc = np.ones((128, 3), np.float32)
    for g in range(3):
        for j in range(2):
            tmpc[64 * j:64 * j + 32, g] = temperature[2 * g + j]
    onesr = np.ones((1, 128), np.float32)

    nc, t = _module()
    rep = dict(wdw=wdw_t, dwd=dwd, wpT=wpT, idf=idf, tmpc=tmpc, onesr=onesr)
    kv_all = k_v @ w_kernel.T                      # [8, 384]
    in_maps = []
    for b in range(8):
        s, tt = kv_all[b, :DIM], kv_all[b, DIM:]
        Ws = Wp * s[None, :]
        bias = Wp @ tt
        wqT = np.zeros((193, 640), np.float32)
        wqT[0:DIM, 0:576] = Ws.T
        wqT[DIM, 0:576] = bias
        xb_ = x[b].reshape(DIM, N)
        m = {"xa": np.ascontiguousarray(xb_[:128]).astype(NPBF16),
             "xb": np.concatenate([xb_[128:], np.ones((1, N), np.float32)], 0).astype(NPBF16),
             "wq1": wqT[0:128].astype(NPBF16),
             "wq2": np.concatenate([wqT[128:192], wqT[192:193]], 0).astype(NPBF16)}
        m.update(rep)
        in_maps.append(m)
    res = run_bass_kernel_spmd(nc, in_maps, core_ids=list(range(8)))
    outs = []
    for b in range(8):
        yA = np.asarray(res.results[b]["yA"])
        yB = np.asarray(res.results[b]["yB"])
        outs.append(np.concatenate([yA, yB], axis=0).reshape(DIM, H, W))
    return np.stack(outs).astype(np.float32)


# revision 5
# speedup vs baseline: 1.3651x; 1.3651x over previous
"""Trainium2 Bass kernel for nn_Attention_35905926595471.

Channel-attention (XCA-style) block, data-parallel over batch: 8 samples on
8 NeuronCores. FiLM is folded into per-sample qkv weights on the host (bias via
a ones-channel in the contraction). qkv 1x1 conv runs on PE in bf16; the 3x3
depthwise conv is staged into zero-padded 130-stride planes built from
self-contained 34-row quarters (halo rows recomputed with 1-row matmuls, so
quarters have no cross dependencies and schedule freely). q/k conv taps run as
4x tensor-scalar muls on DVE/ACT/Pool with adds on DVE plus compute-capable
DMA (accum_op=add); v conv taps run as diagonal-weight matmuls accumulating in
PSUM. Grams come from DMA-transposed conv outputs; v-ot3 quarters interleave
into the q/k phase to keep PE busy; softmax is batched to limit ACT table
swaps; the attention map folds into the output projection before the final
matmul, staged to bf16 and upcast by the output DMA.
"""
import numpy as np
from contextlib import ExitStack

import concourse.bacc as bacc
import concourse.bass as bass
import concourse.mybir as mybir
from concourse import tile
from concourse.bass_utils import run_bass_kernel_spmd

F32 = mybir.dt.float32
BF16 = mybir.dt.bfloat16
NPBF16 = mybir.dt.np(BF16)
AL = mybir.AluOpType
AX = mybir.AxisListType
AF = mybir.ActivationFunctionType

DIM, HEADS, H, W = 192, 6, 128, 128
HD = DIM // HEADS          # 32
N = H * W                  # 16384
CH = 2048                  # px per chunk (16 rows)
SW = W + 2                 # padded row stride 130
QROWS = 34                 # quarter plane rows: 32 image rows + 2 halo
PROWS = 18                 # ot4 pair plane rows: 16 image rows + 2 halo


def _perm():
    perm = []
    for t in range(3):
        for h in (2 * t, 2 * t + 1):
            perm += list(range(h * HD, (h + 1) * HD))
            perm += list(range(DIM + h * HD, DIM + (h + 1) * HD))
    perm += list(range(2 * DIM, 3 * DIM))
    return np.array(perm)


def _emit(nc, t):
    with ExitStack() as ctx:
        tc = ctx.enter_context(tile.TileContext(nc))
        sb = ctx.enter_context(tc.tile_pool(name="sb", bufs=1))
        plp = ctx.enter_context(tc.tile_pool(name="planes", bufs=4))
        ppp = ctx.enter_context(tc.tile_pool(name="pplanes", bufs=2))
        acp = ctx.enter_context(tc.tile_pool(name="accq", bufs=2))
        qtp = ctx.enter_context(tc.tile_pool(name="qkt", bufs=2))
        sc2 = ctx.enter_context(tc.tile_pool(name="scr2", bufs=2))
        sc1 = ctx.enter_context(tc.tile_pool(name="scr1", bufs=1))
        ysp = ctx.enter_context(tc.tile_pool(name="ys", bufs=2))
        v4p = ctx.enter_context(tc.tile_pool(name="v4", bufs=2))
        xbp = ctx.enter_context(tc.tile_pool(name="xb2", bufs=4))
        xhp = ctx.enter_context(tc.tile_pool(name="xh", bufs=2))
        pq = ctx.enter_context(tc.tile_pool(name="pq", bufs=3, space=bass.MemorySpace.PSUM))
        pb = ctx.enter_context(tc.tile_pool(name="pb", bufs=2, space=bass.MemorySpace.PSUM))

        # ---- resident tensors ----
        xs1 = sb.tile([128, N], BF16, tag="xs1", name="xs1")
        vsb3 = sb.tile([128, N], BF16, tag="vsb3", name="vsb3")
        wq1 = sb.tile([128, 640], BF16, tag="wq1", name="wq1")
        wq2 = sb.tile([65, 640], BF16, tag="wq2", name="wq2")
        wdw = sb.tile([128, 45], F32, tag="wdw", name="wdw")
        dwd = sb.tile([128, 18 * 128], BF16, tag="dwd", name="dwd")
        wpT = [sb.tile([128, DIM], F32, tag="wpT0", name="wpT0"),
               sb.tile([64, DIM], F32, tag="wpT1", name="wpT1")]
        idf = sb.tile([128, 128], F32, tag="idf", name="idf")
        tmpc = sb.tile([128, 3], F32, tag="tmpc", name="tmpc")
        onesr = sb.tile([1, 128], F32, tag="onesr", name="onesr")
        sm = sb.tile([128, 16], F32, tag="sm", name="sm")
        Lsb = [sb.tile([128, 128], F32, tag=f"L{g}", name=f"L{g}") for g in range(3)]
        dscr = sb.tile([128, 128], F32, tag="dscr", name="dscr")
        nrow = [sb.tile([1, 128], F32, tag=f"nrow{g}", name=f"nrow{g}") for g in range(3)]
        Asb = [sb.tile([128, DIM], F32, tag="A0", name="A0"), sb.tile([64, DIM], F32, tag="A1", name="A1")]
        w2t1 = sb.tile([128, DIM], BF16, tag="w2t1", name="w2t1")
        w2t2 = sb.tile([128, DIM], BF16, tag="w2t2", name="w2t2")

        nc.sync.dma_start(wq1[:], t["wq1"].ap()[:, :])
        nc.sync.dma_start(wq2[:], t["wq2"].ap()[:, :])
        nc.sync.dma_start(wdw[:], t["wdw"].ap()[:, :])
        nc.sync.dma_start(dwd[:], t["dwd"].ap()[:, :])
        nc.sync.dma_start(wpT[0][:], t["wpT"].ap()[0:128, :])
        nc.sync.dma_start(wpT[1][:], t["wpT"].ap()[128:192, :])
        nc.sync.dma_start(idf[:], t["idf"].ap()[:, :])
        nc.sync.dma_start(tmpc[:], t["tmpc"].ap()[:, :])
        nc.sync.dma_start(onesr[:], t["onesr"].ap()[:, :])
        for ci in range(8):
            nc.sync.dma_start(xs1[:, ci * CH:(ci + 1) * CH], t["xa"].ap()[:, ci * CH:(ci + 1) * CH])

        def wcol(g, tap):
            return wdw[0:128, g * 9 + tap:g * 9 + tap + 1]

        def slab(c):
            xb = xbp.tile([65, CH], BF16, tag="xb2", name="xb2")
            nc.sync.dma_start(xb[:], t["xb"].ap()[:, c * CH:(c + 1) * CH])
            return xb

        def hslab(px0):
            xh = xhp.tile([65, 128], BF16, tag="xh", name="xh")
            nc.sync.dma_start(xh[:], t["xb"].ap()[:, px0:px0 + 128])
            return xh

        def mm_row(ps_region, g, px0, xh, base=0, cn=128):
            """1-row (128 px) qkv matmul into a psum region."""
            c0 = g * 128
            nc.tensor.matmul(ps_region, wq1[:, c0:c0 + cn], xs1[:, px0:px0 + 128],
                             start=True, stop=False)
            nc.tensor.matmul(ps_region, wq2[0:65, c0:c0 + cn], xh[0:65, 0:128],
                             start=False, stop=True)

        # ---------------- self-contained quarter for planar ots (g=0..3) --------
        def emit_quarter(g, q, taps_fn):
            pl = plp.tile([128, QROWS * SW], BF16, tag="pl", name="pl")
            s3 = pl[:].rearrange("p (r c) -> p r c", c=SW)
            nc.gpsimd.memset(s3[:, :, 0:1], 0.0)
            nc.gpsimd.memset(s3[:, :, 129:130], 0.0)
            if q == 0:
                nc.gpsimd.memset(s3[:, 0:1, :], 0.0)
            if q == 3:
                nc.gpsimd.memset(s3[:, 33:34, :], 0.0)
            # halo rows via 1-row matmuls
            if q > 0 or q < 3:
                hp = pq.tile([128, 512], F32, tag="mm", name="hp")
                h3 = hp[:].rearrange("p (r c) -> p r c", c=W)
                if q > 0:
                    px0 = (32 * q - 1) * 128
                    mm_row(hp[:, 0:128], g, px0, hslab(px0))
                    nc.scalar.activation(s3[:, 0:1, 1:129], h3[:, 0:1, :], AF.Identity, scale=1.0)
                if q < 3:
                    px0 = (32 * q + 32) * 128
                    mm_row(hp[:, 128:256], g, px0, hslab(px0))
                    nc.scalar.activation(s3[:, 33:34, 1:129], h3[:, 1:2, :], AF.Identity, scale=1.0)
            # main chunks
            for c in (2 * q, 2 * q + 1):
                xb = slab(c)
                lb = 16 * (c % 2)
                for k in range(4):
                    px = c * CH + 512 * k
                    ps = pq.tile([128, 512], F32, tag="mm", name="mm")
                    nc.tensor.matmul(ps[:], wq1[:, g * 128:(g + 1) * 128],
                                     xs1[:, px:px + 512], start=True, stop=False)
                    nc.tensor.matmul(ps[:], wq2[0:65, g * 128:(g + 1) * 128],
                                     xb[0:65, 512 * k:512 * k + 512], start=False, stop=True)
                    p3 = ps[:].rearrange("p (r c) -> p r c", c=W)
                    r0 = lb + 1 + 4 * k
                    nc.scalar.activation(s3[:, r0:r0 + 4, 1:129], p3[:], AF.Identity, scale=1.0)
            taps_fn(g, 2 * q, s3, 0)
            taps_fn(g, 2 * q + 1, s3, 16)

        # ---- q/k taps: DVE muls/adds + ACT mul + Pool muls + DMA CCE adds ----
        def qk_taps(g, c, s3, lb):
            def view(tap):
                dy, dx = tap // 3, tap % 3
                return s3[:, lb + dy:lb + dy + 16, dx:dx + 128]

            def v3(tl):
                return tl[:].rearrange("p (r c) -> p r c", c=W)

            acc = acp.tile([128, CH], BF16, tag="acc", name="acc")
            nc.vector.tensor_scalar_mul(v3(acc), view(4), wcol(g, 4))
            pA = sc2.tile([128, CH], BF16, tag="sA", name="sA")
            pB = sc1.tile([128, CH], BF16, tag="sB", name="sB")
            pC = sc2.tile([128, CH], BF16, tag="sC", name="sC")
            pD = sc2.tile([128, CH], BF16, tag="sD", name="sD")
            pE = sc1.tile([128, CH], BF16, tag="sE", name="sE")
            nc.scalar.mul(v3(pA), view(0), wcol(g, 0))                # ACT
            nc.gpsimd.tensor_scalar_mul(v3(pB), view(2), wcol(g, 2))  # Pool
            nc.gpsimd.tensor_scalar_mul(v3(pC), view(6), wcol(g, 6))  # Pool
            nc.vector.tensor_scalar_mul(v3(pD), view(1), wcol(g, 1))  # DVE
            nc.vector.tensor_scalar_mul(v3(pE), view(3), wcol(g, 3))  # DVE
            nc.gpsimd.dma_start(pA[:], pB[:], accum_op=AL.add)        # CCE: t0+t2
            nc.gpsimd.dma_start(pD[:], pE[:], accum_op=AL.add)        # CCE: t1+t3
            pB2 = sc1.tile([128, CH], BF16, tag="sB", name="sB")
            pE2 = sc1.tile([128, CH], BF16, tag="sE", name="sE")
            nc.vector.tensor_scalar_mul(v3(pB2), view(5), wcol(g, 5))  # DVE
            nc.vector.tensor_scalar_mul(v3(pE2), view(7), wcol(g, 7))  # DVE
            nc.vector.tensor_add(pB2[:], pB2[:], pE2[:])              # t5+t7
            pE3 = sc1.tile([128, CH], BF16, tag="sE", name="sE")
            nc.vector.tensor_scalar_mul(v3(pE3), view(8), wcol(g, 8))  # DVE
            nc.gpsimd.dma_start(pC[:], pE3[:], accum_op=AL.add)       # CCE: t6+t8
            nc.vector.tensor_add(acc[:], acc[:], pB2[:])
            nc.vector.tensor_add(acc[:], acc[:], pD[:])
            nc.vector.tensor_add(acc[:], acc[:], pA[:])
            nc.vector.tensor_add(acc[:], acc[:], pC[:])

            qt = qtp.tile([128, CH], BF16, tag="qt", name="qt")
            qt3 = qt[:].rearrange("p (b j) -> p b j", j=128)
            nc.sync.dma_start_transpose(qt3, acc[:])
            gram = grams[g]
            for b in range(16):
                nc.tensor.matmul(gram[:], qt[:, b * 128:(b + 1) * 128],
                                 qt[:, b * 128:(b + 1) * 128],
                                 start=(c == 0 and b == 0), stop=(c == 7 and b == 15))

        # ---- v ot3 taps: 9 diagonal-weight matmuls per 1024-px PSUM ----
        def v3_taps(g, c, s3, lb):
            for k in range(2):
                tp = pb.tile([128, 1024], F32, tag="vt", name="vt")
                t3_ = tp[:].rearrange("p (r c) -> p r c", c=W)
                for q in range(2):
                    for tap in range(9):
                        dy, dx = tap // 3, tap % 3
                        r = lb + 8 * k + 4 * q + dy
                        nc.tensor.matmul(t3_[:, 4 * q:4 * q + 4, :], dwd[:, tap * 128:(tap + 1) * 128],
                                         s3[:, r:r + 4, dx:dx + 128], start=(tap == 0), stop=(tap == 8))
                nc.scalar.activation(vsb3[0:128, c * CH + k * 1024:c * CH + k * 1024 + 1024],
                                     tp[:], AF.Identity, scale=1.0)

        # ================= interleaved qk + v-ot3 phase =================
        grams = {}
        v3q = iter(range(4))
        ILV = {(0, 1): True, (1, 0): True, (1, 2): True, (2, 0): True}
        for g in range(3):
            grams[g] = pb.tile([128, 128], F32, tag="gr", bufs=1, name=f"gram{g}")
            for q in range(4):
                emit_quarter(g, q, qk_taps)
                if ILV.get((g, q)):
                    nq = next(v3q, None)
                    if nq is not None:
                        emit_quarter(3, nq, v3_taps)
            nc.scalar.copy(Lsb[g][:], grams[g][:])
        for nq in v3q:
            emit_quarter(3, nq, v3_taps)

        # ================= norms + softmax (batched) =================
        for g in range(3):
            L = Lsb[g]
            dcol = sm[:, 9:10]
            scrc = sm[:, 10:11]
            dsc = sm[:, 11:12]
            nc.vector.tensor_mul(dscr[:], L[:], idf[:])
            nc.vector.reduce_sum(dcol, dscr[:], axis=AX.X)
            nc.scalar.sqrt(scrc, dcol)
            nc.vector.tensor_scalar_max(scrc, scrc, 1e-12)
            nc.vector.reciprocal(dsc, scrc)
            rs = sm[:, 12:13]
            nc.vector.tensor_mul(rs, dsc, tmpc[:, g:g + 1])
            pt = pq.tile([128, 512], F32, tag="mm", name="pt")
            nc.tensor.transpose(pt[0:1, 0:128], dsc, idf[:])
            nc.scalar.copy(nrow[g][:], pt[0:1, 0:128])
            pt2 = pq.tile([128, 512], F32, tag="mm", name="pt2")
            nc.tensor.matmul(pt2[:, 0:128], onesr[:], nrow[g][:], start=True, stop=True)
            nc.vector.tensor_scalar_mul(L[:], L[:], rs)
            nc.vector.tensor_mul(L[:], L[:], pt2[:, 0:128])
            for j in range(2):
                P0, K0 = 64 * j, 64 * j + 32
                mx = sm[P0:P0 + 32, 14:15]
                nc.vector.reduce_max(mx, L[P0:P0 + 32, K0:K0 + 32], axis=AX.X)
                nc.vector.tensor_scalar_sub(L[P0:P0 + 32, K0:K0 + 32], L[P0:P0 + 32, K0:K0 + 32], mx)
                nc.scalar.activation(L[P0:P0 + 32, K0:K0 + 32], L[P0:P0 + 32, K0:K0 + 32], AF.Exp)
                nc.vector.reduce_sum(mx, L[P0:P0 + 32, K0:K0 + 32], axis=AX.X)
                nc.vector.reciprocal(mx, mx)
                nc.vector.tensor_scalar_mul(L[P0:P0 + 32, K0:K0 + 32], L[P0:P0 + 32, K0:K0 + 32], mx)

        # ================= A_bd + W2T =================
        nc.gpsimd.memset(Asb[0][:], 0.0)
        nc.gpsimd.memset(Asb[1][:], 0.0)
        for h in range(HEADS):
            g, j = h // 2, h % 2
            src = Lsb[g][64 * j:64 * j + 32, 64 * j + 32:64 * j + 64]
            dst_t = Asb[0] if h < 4 else Asb[1]
            dp = 32 * (h % 4)
            dst = dst_t[dp:dp + 32, 32 * h:32 * h + 32]
            if dp == 64 * j:
                nc.vector.tensor_copy(dst, src)
            else:
                nc.sync.dma_start(dst, src)
        for dt_ in range(2):
            c0, cn = dt_ * 128, (128 if dt_ == 0 else 64)
            ps = pq.tile([128, 512], F32, tag="mm", name="w2")
            nc.tensor.matmul(ps[0:cn, 0:DIM], Asb[0][:, c0:c0 + cn], wpT[0][:], start=True, stop=False)
            nc.tensor.matmul(ps[0:cn, 0:DIM], Asb[1][:, c0:c0 + cn], wpT[1][:], start=False, stop=True)
            if dt_ == 0:
                nc.scalar.copy(w2t1[:], ps[0:128, 0:DIM])
            else:
                nc.scalar.copy(w2t2[0:64, :], ps[0:64, 0:DIM])
        nc.sync.dma_start(w2t2[64:128, :], w2t2[0:64, :])

        # ================= v ot4 (pairs on partition halves) + y =================
        def emit_y(ci, v4):
            px = 1024 * ci
            b2 = 64 * ((ci // 2) % 2)
            loc = 1024 * (ci % 2)
            for oT, (o0, on) in enumerate([(0, 128), (128, 64)]):
                yp = pb.tile([128, 1024], F32, tag="vt", name="yp")
                for q in range(2):
                    nc.tensor.matmul(yp[0:on, q * 512:(q + 1) * 512], w2t1[:, o0:o0 + on],
                                     vsb3[0:128, px + q * 512:px + q * 512 + 512], start=True, stop=False)
                    nc.tensor.matmul(yp[0:on, q * 512:(q + 1) * 512], w2t2[b2:b2 + 64, o0:o0 + on],
                                     v4[b2:b2 + 64, loc + q * 512:loc + q * 512 + 512], start=False, stop=True)
                ys = ysp.tile([128, 1024], BF16, tag=("ysA" if oT == 0 else "ysB"), name="ys")
                nc.scalar.copy(ys[0:on, :], yp[0:on, :])
                dst = t["yA"] if oT == 0 else t["yB"]
                nc.gpsimd.dma_start(dst.ap()[:, px:px + 1024], ys[0:on, :])

        for p in range(4):
            pp = ppp.tile([128, PROWS * SW], BF16, tag="pp", name="pp")
            s3 = pp[:].rearrange("p (r c) -> p r c", c=SW)
            nc.gpsimd.memset(s3[:, :, 0:1], 0.0)
            nc.gpsimd.memset(s3[:, :, 129:130], 0.0)
            if p == 0:
                nc.gpsimd.memset(s3[0:64, 0:1, :], 0.0)
            if p == 3:
                nc.gpsimd.memset(s3[64:128, 17:18, :], 0.0)
            # halo rows: (partition half, plane row, image row)
            hp = pq.tile([128, 512], F32, tag="mm", name="hp4")
            h3 = hp[:].rearrange("p (r c) -> p r c", c=W)
            halos = []
            if p > 0:
                halos.append((0, 0, 32 * p - 1, 0))
            halos.append((0, 17, 32 * p + 16, 1))
            halos.append((64, 0, 32 * p + 15, 2))
            if p < 3:
                halos.append((64, 17, 32 * p + 32, 3))
            for (pb0, prow, irow, slot) in halos:
                px0 = irow * 128
                mm_row(hp[pb0:pb0 + 64, slot * 128:(slot + 1) * 128], 4, px0, hslab(px0), cn=64)
                nc.scalar.activation(s3[pb0:pb0 + 64, prow:prow + 1, 1:129],
                                     h3[pb0:pb0 + 64, slot:slot + 1, :], AF.Identity, scale=1.0)
            xbe, xbo = slab(2 * p), slab(2 * p + 1)
            for k in range(4):
                pse = pq.tile([128, 512], F32, tag="mm", name="mm4")
                pxe = (2 * p) * CH + 512 * k
                pxo = (2 * p + 1) * CH + 512 * k
                nc.tensor.matmul(pse[0:64, :], wq1[:, 512:576], xs1[:, pxe:pxe + 512], start=True, stop=False)
                nc.tensor.matmul(pse[0:64, :], wq2[0:65, 512:576], xbe[0:65, 512 * k:512 * k + 512],
                                 start=False, stop=True)
                nc.tensor.matmul(pse[64:128, :], wq1[:, 512:576], xs1[:, pxo:pxo + 512], start=True, stop=False)
                nc.tensor.matmul(pse[64:128, :], wq2[0:65, 512:576], xbo[0:65, 512 * k:512 * k + 512],
                                 start=False, stop=True)
                p3 = pse[:].rearrange("p (r c) -> p r c", c=W)
                nc.scalar.activation(s3[:, 1 + 4 * k:5 + 4 * k, 1:129], p3[:], AF.Identity, scale=1.0)
            v4 = v4p.tile([128, CH], BF16, tag="v4", name="v4")
            for k in range(2):
                tp = pb.tile([128, 1024], F32, tag="vt", name="vt4")
                t3_ = tp[:].rearrange("p (r c) -> p r c", c=W)
                for q in range(2):
                    for tap in range(9):
                        dy, dx = tap // 3, tap % 3
                        r = 8 * k + 4 * q + dy
                        nc.tensor.matmul(t3_[:, 4 * q:4 * q + 4, :], dwd[:, (9 + tap) * 128:(10 + tap) * 128],
                                         s3[:, r:r + 4, dx:dx + 128], start=(tap == 0), stop=(tap == 8))
                nc.scalar.activation(v4[0:64, k * 1024:k * 1024 + 1024], tp[0:64, :], AF.Identity, scale=1.0)
                nc.scalar.activation(v4[64:128, k * 1024:k * 1024 + 1024], tp[64:128, :], AF.Identity, scale=1.0)
            for ci in range(4 * p, 4 * p + 4):
                emit_y(ci, v4)


_CACHE = {}


def _module():
    if "nc" in _CACHE:
        return _CACHE["nc"], _CACHE["t"]
    nc = bacc.Bacc("TRN2", target_bir_lowering=False, debug=False)
    t = {
        "xa": nc.dram_tensor("xa", [128, N], BF16, kind="ExternalInput"),
        "xb": nc.dram_tensor("xb", [65, N], BF16, kind="ExternalInput"),
        "wq1": nc.dram_tensor("wq1", [128, 640], BF16, kind="ExternalInput"),
        "wq2": nc.dram_tensor("wq2", [65, 640], BF16, kind="ExternalInput"),
        "wdw": nc.dram_tensor("wdw", [128, 45], F32, kind="ExternalInput"),
        "dwd": nc.dram_tensor("dwd", [128, 18 * 128], BF16, kind="ExternalInput"),
        "wpT": nc.dram_tensor("wpT", [192, DIM], F32, kind="ExternalInput"),
        "idf": nc.dram_tensor("idf", [128, 128], F32, kind="ExternalInput"),
        "tmpc": nc.dram_tensor("tmpc", [128, 3], F32, kind="ExternalInput"),
        "onesr": nc.dram_tensor("onesr", [1, 128], F32, kind="ExternalInput"),
        "yA": nc.dram_tensor("yA", [128, N], F32, kind="ExternalOutput"),
        "yB": nc.dram_tensor("yB", [64, N], F32, kind="ExternalOutput"),
    }
    _emit(nc, t)
    nc.compile()
    _CACHE["nc"], _CACHE["t"] = nc, t
    return nc, t


def kernel(x, k_v, w_kernel, w_qkv, w_dw, w_proj, temperature):
    x = np.asarray(x, np.float32)
    k_v = np.asarray(k_v, np.float32)
    w_kernel = np.asarray(w_kernel, np.float32)
    w_qkv = np.asarray(w_qkv, np.float32)
    w_dw = np.asarray(w_dw, np.float32)
    w_proj = np.asarray(w_proj, np.float32)
    temperature = np.asarray(temperature, np.float32).reshape(-1)

    perm = _perm()
    Wp = w_qkv[perm]                               # [576, 192]
    wdw_p = w_dw.reshape(3 * DIM, 9)[perm]         # [576, 9]
    wdw_t = np.zeros((128, 45), np.float32)
    for g in range(4):
        wdw_t[:, g * 9:(g + 1) * 9] = wdw_p[g * 128:(g + 1) * 128]
    wdw_t[0:64, 36:45] = wdw_p[512:576]
    wdw_t[64:128, 36:45] = wdw_p[512:576]
    dwd = np.zeros((128, 18 * 128), np.float32)
    for tap in range(9):
        dwd[:, tap * 128:(tap + 1) * 128][np.arange(128), np.arange(128)] = wdw_p[384 + np.arange(128), tap]
        blk = dwd[:, (9 + tap) * 128:(10 + tap) * 128]
        blk[np.arange(64), np.arange(64)] = wdw_p[512 + np.arange(64), tap]
        blk[64 + np.arange(64), 64 + np.arange(64)] = wdw_p[512 + np.arange(64), tap]
    dwd = dwd.astype(NPBF16)

    wpT = np.ascontiguousarray(w_proj.T)
    idf = np.eye(128, dtype=np.float32)
    tmpc = np.ones((128, 3), np.float32)
    for g in range(3):
        for j in range(2):
            tmpc[64 * j:64 * j + 32, g] = temperature[2 * g + j]
    onesr = np.ones((1, 128), np.float32)

    nc, t = _module()
    rep = dict(wdw=wdw_t, dwd=dwd, wpT=wpT, idf=idf, tmpc=tmpc, onesr=onesr)
    kv_all = k_v @ w_kernel.T                      # [8, 384]
    in_maps = []
    for b in range(8):
        s, tt = kv_all[b, :DIM], kv_all[b, DIM:]
        Ws = Wp * s[None, :]
        bias = Wp @ tt
        wqT = np.zeros((193, 640), np.float32)
        wqT[0:DIM, 0:576] = Ws.T
        wqT[DIM, 0:576] = bias
        xb_ = x[b].reshape(DIM, N)
        m = {"xa": np.ascontiguousarray(xb_[:128]).astype(NPBF16),
             "xb": np.concatenate([xb_[128:], np.ones((1, N), np.float32)], 0).astype(NPBF16),
             "wq1": wqT[0:128].astype(NPBF16),
             "wq2": np.concatenate([wqT[128:192], wqT[192:193]], 0).astype(NPBF16)}
        m.update(rep)
        in_maps.append(m)
    res = run_bass_kernel_spmd(nc, in_maps, core_ids=list(range(8)))
    outs = []
    for b in range(8):
        yA = np.asarray(res.results[b]["yA"])
        yB = np.asarray(res.results[b]["yB"])
        outs.append(np.concatenate([yA, yB], axis=0).reshape(DIM, H, W))
    return np.stack(outs).astype(np.float32)


# revision 13
# speedup vs baseline: 1.7137x; 1.2553x over previous
"""Trainium2 Bass kernel for nn_Attention_35905926595471.

Channel-attention (XCA-style) block, data-parallel over batch: 8 samples on
8 NeuronCores. FiLM is folded into per-sample qkv weights on the host (bias via
a ones-channel in the contraction). qkv 1x1 conv runs on PE in bf16; the 3x3
depthwise conv is staged into zero-padded 130-stride planes built from
self-contained 34-row quarters (halo rows recomputed with 1-row matmuls, so
quarters have no cross dependencies and schedule freely). q/k conv taps run as
4x tensor-scalar muls on DVE/ACT/Pool with adds on DVE plus compute-capable
DMA (accum_op=add); v conv taps run as diagonal-weight matmuls accumulating in
PSUM. Grams come from DMA-transposed conv outputs; v-ot3 quarters interleave
into the q/k phase to keep PE busy; softmax is batched to limit ACT table
swaps; the attention map folds into the output projection before the final
matmul, staged to bf16 and upcast by the output DMA.
"""
import numpy as np
from contextlib import ExitStack

import concourse.bacc as bacc
import concourse.bass as bass
import concourse.mybir as mybir
from concourse import tile
from concourse.bass_utils import run_bass_kernel_spmd

F32 = mybir.dt.float32
BF16 = mybir.dt.bfloat16
NPBF16 = mybir.dt.np(BF16)
AL = mybir.AluOpType
AX = mybir.AxisListType
AF = mybir.ActivationFunctionType

DIM, HEADS, H, W = 192, 6, 128, 128
HD = DIM // HEADS          # 32
N = H * W                  # 16384
CH = 2048                  # px per chunk (16 rows)
SW = W + 2                 # padded row stride 130
QROWS = 34                 # quarter plane rows: 32 image rows + 2 halo
PROWS = 18                 # ot4 pair plane rows: 16 image rows + 2 halo


def _perm():
    perm = []
    for t in range(3):
        for h in (2 * t, 2 * t + 1):
            perm += list(range(h * HD, (h + 1) * HD))
            perm += list(range(DIM + h * HD, DIM + (h + 1) * HD))
    perm += list(range(2 * DIM, 3 * DIM))
    return np.array(perm)


def _emit(nc, t):
    with ExitStack() as ctx:
        tc = ctx.enter_context(tile.TileContext(nc))
        sb = ctx.enter_context(tc.tile_pool(name="sb", bufs=1))
        plp = ctx.enter_context(tc.tile_pool(name="planes", bufs=3))
        ppp = ctx.enter_context(tc.tile_pool(name="pplanes", bufs=2))
        acp = ctx.enter_context(tc.tile_pool(name="accq", bufs=3))
        xbp = ctx.enter_context(tc.tile_pool(name="xb2", bufs=3))
        xhp = ctx.enter_context(tc.tile_pool(name="xh", bufs=2))
        qtp = ctx.enter_context(tc.tile_pool(name="qkt", bufs=2))
        scr = ctx.enter_context(tc.tile_pool(name="scr", bufs=1))
        ysp = ctx.enter_context(tc.tile_pool(name="ys", bufs=2))
        v4p = ctx.enter_context(tc.tile_pool(name="v4", bufs=2))
        pq = ctx.enter_context(tc.tile_pool(name="pq", bufs=3, space=bass.MemorySpace.PSUM))
        pb = ctx.enter_context(tc.tile_pool(name="pb", bufs=2, space=bass.MemorySpace.PSUM))

        # ---- resident tensors ----
        xs1 = sb.tile([128, N], BF16, tag="xs1", name="xs1")
        vsb3 = sb.tile([128, N], BF16, tag="vsb3", name="vsb3")
        wq1 = sb.tile([128, 640], BF16, tag="wq1", name="wq1")
        wq2 = sb.tile([65, 640], BF16, tag="wq2", name="wq2")
        wdw = sb.tile([128, 45], F32, tag="wdw", name="wdw")
        dwd = sb.tile([128, 22 * 128], BF16, tag="dwd", name="dwd")
        wpT = [sb.tile([128, DIM], F32, tag="wpT0", name="wpT0"),
               sb.tile([64, DIM], F32, tag="wpT1", name="wpT1")]
        idf = sb.tile([128, 128], F32, tag="idf", name="idf")
        tmpc = sb.tile([128, 3], F32, tag="tmpc", name="tmpc")
        onesr = sb.tile([1, 128], F32, tag="onesr", name="onesr")
        sm = sb.tile([128, 16], F32, tag="sm", name="sm")
        Lsb = [sb.tile([128, 128], F32, tag=f"L{g}", name=f"L{g}") for g in range(3)]
        dscr = sb.tile([128, 128], F32, tag="dscr", name="dscr")
        nrow = [sb.tile([1, 128], F32, tag=f"nrow{g}", name=f"nrow{g}") for g in range(3)]
        Asb = [sb.tile([128, DIM], F32, tag="A0", name="A0"), sb.tile([64, DIM], F32, tag="A1", name="A1")]
        w2t1 = sb.tile([128, DIM], BF16, tag="w2t1", name="w2t1")
        w2t2 = sb.tile([128, DIM], BF16, tag="w2t2", name="w2t2")

        nc.sync.dma_start(wq1[:], t["wq1"].ap()[:, :])
        nc.sync.dma_start(wq2[:], t["wq2"].ap()[:, :])
        nc.sync.dma_start(wdw[:], t["wdw"].ap()[:, :])
        for ci in range(3):
            nc.sync.dma_start(xs1[:, ci * CH:(ci + 1) * CH], t["xa"].ap()[:, ci * CH:(ci + 1) * CH])
        nc.sync.dma_start(dwd[:], t["dwd"].ap()[:, :])
        for ci in range(3, 8):
            nc.sync.dma_start(xs1[:, ci * CH:(ci + 1) * CH], t["xa"].ap()[:, ci * CH:(ci + 1) * CH])
        nc.sync.dma_start(wpT[0][:], t["wpT"].ap()[0:128, :])
        nc.sync.dma_start(wpT[1][:], t["wpT"].ap()[128:192, :])
        nc.sync.dma_start(idf[:], t["idf"].ap()[:, :])
        nc.sync.dma_start(tmpc[:], t["tmpc"].ap()[:, :])
        nc.sync.dma_start(onesr[:], t["onesr"].ap()[:, :])

        def wcol(g, tap):
            return wdw[0:128, g * 9 + tap:g * 9 + tap + 1]

        slab_cache = {}

        def slab_prefetch(c):
            xb = xbp.tile([65, CH], BF16, tag="xb2", name="xb2")
            nc.sync.dma_start(xb[:], t["xb"].ap()[:, c * CH:(c + 1) * CH])
            slab_cache[c] = xb

        def slab(c):
            if c in slab_cache:
                return slab_cache.pop(c)
            xb = xbp.tile([65, CH], BF16, tag="xb2", name="xb2")
            nc.sync.dma_start(xb[:], t["xb"].ap()[:, c * CH:(c + 1) * CH])
            return xb

        slab_prefetch(0)
        slab_prefetch(1)

        def hslab(px0):
            xh = xhp.tile([65, 128], BF16, tag="xh", name="xh")
            nc.sync.dma_start(xh[:], t["xb"].ap()[:, px0:px0 + 128])
            return xh

        def mm_row(ps_region, g, px0, xh, cn=128):
            """1-row (128 px) qkv matmul into a psum region."""
            c0 = g * 128
            nc.tensor.matmul(ps_region, wq1[:, c0:c0 + cn], xs1[:, px0:px0 + 128],
                             start=True, stop=False)
            nc.tensor.matmul(ps_region, wq2[0:65, c0:c0 + cn], xh[0:65, 0:128],
                             start=False, stop=True)

        # ---------------- self-contained quarter for planar ots (g=0..3) --------
        def emit_quarter(g, q, taps_fn):
            pl = plp.tile([128, QROWS * SW], BF16, tag="pl", name="pl")
            s3 = pl[:].rearrange("p (r c) -> p r c", c=SW)
            nc.gpsimd.memset(s3[:, :, 0:1], 0.0)
            nc.gpsimd.memset(s3[:, :, 129:130], 0.0)
            if q == 0:
                nc.gpsimd.memset(s3[:, 0:1, :], 0.0)
            if q == 3:
                nc.gpsimd.memset(s3[:, 33:34, :], 0.0)
            # halo rows via 1-row matmuls
            if q > 0 or q < 3:
                hp = pq.tile([128, 512], F32, tag="mm", name="hp")
                h3 = hp[:].rearrange("p (r c) -> p r c", c=W)
                if q > 0:
                    px0 = (32 * q - 1) * 128
                    mm_row(hp[:, 0:128], g, px0, hslab(px0))
                    nc.scalar.activation(s3[:, 0:1, 1:129], h3[:, 0:1, :], AF.Identity, scale=1.0)
                if q < 3:
                    px0 = (32 * q + 32) * 128
                    mm_row(hp[:, 128:256], g, px0, hslab(px0))
                    nc.scalar.activation(s3[:, 33:34, 1:129], h3[:, 1:2, :], AF.Identity, scale=1.0)
            # main chunks
            for c in (2 * q, 2 * q + 1):
                xb = slab(c)
                lb = 16 * (c % 2)
                for k in range(4):
                    px = c * CH + 512 * k
                    ps = pq.tile([128, 512], F32, tag="mm", name="mm")
                    nc.tensor.matmul(ps[:], wq1[:, g * 128:(g + 1) * 128],
                                     xs1[:, px:px + 512], start=True, stop=False)
                    nc.tensor.matmul(ps[:], wq2[0:65, g * 128:(g + 1) * 128],
                                     xb[0:65, 512 * k:512 * k + 512], start=False, stop=True)
                    p3 = ps[:].rearrange("p (r c) -> p r c", c=W)
                    r0 = lb + 1 + 4 * k
                    nc.scalar.activation(s3[:, r0:r0 + 4, 1:129], p3[:], AF.Identity, scale=1.0)
            taps_fn(g, 2 * q, s3, 0)
            taps_fn(g, 2 * q + 1, s3, 16)

        # ---- q/k taps: split into mul-part / add-part for pipelining ----
        def qk_taps_mul(g, c, s3, lb):
            def view(tap):
                dy, dx = tap // 3, tap % 3
                return s3[:, lb + dy:lb + dy + 16, dx:dx + 128]

            def v3(tl):
                return tl[:].rearrange("p (r c) -> p r c", c=W)

            pe4 = (g, c) in PE4
            acc = acp.tile([128, CH], BF16, tag="acc", name="acc")
            nc.vector.tensor_scalar_mul(v3(acc), view(4), wcol(g, 4))
            st = {"g": g, "c": c, "acc": acc, "pe4": pe4}
            if pe4:
                pX = scr.tile([128, CH], BF16, tag="s0", name="s0")
                for k in range(2):
                    tp = pb.tile([128, 1024], F32, tag="vt", name="qt4")
                    t3_ = tp[:].rearrange("p (r c) -> p r c", c=W)
                    for q in range(2):
                        for j, tap in enumerate((0, 2, 6, 8)):
                            dy, dx = tap // 3, tap % 3
                            r = lb + 8 * k + 4 * q + dy
                            nc.tensor.matmul(t3_[:, 4 * q:4 * q + 4, :],
                                             dwd[:, (18 + j) * 128:(19 + j) * 128],
                                             s3[:, r:r + 4, dx:dx + 128],
                                             start=(j == 0), stop=(j == 3))
                    nc.scalar.activation(pX[:, k * 1024:k * 1024 + 1024], tp[:], AF.Identity, scale=1.0)
                pD = scr.tile([128, CH], BF16, tag="s3", name="s3")
                pE = scr.tile([128, CH], BF16, tag="s4", name="s4")
                nc.vector.tensor_scalar_mul(v3(pD), view(1), wcol(g, 1))
                nc.vector.tensor_scalar_mul(v3(pE), view(3), wcol(g, 3))
                nc.gpsimd.dma_start(pD[:], pE[:], accum_op=AL.add)        # CCE: t1+t3
                pB2 = scr.tile([128, CH], BF16, tag="s5", name="s5")
                pE2 = scr.tile([128, CH], BF16, tag="s6", name="s6")
                nc.vector.tensor_scalar_mul(v3(pB2), view(5), wcol(g, 5))
                nc.vector.tensor_scalar_mul(v3(pE2), view(7), wcol(g, 7))
                nc.vector.tensor_add(pB2[:], pB2[:], pE2[:])              # t5+t7
                st.update(pX=pX, pD=pD, pB2=pB2)
            else:
                pA = scr.tile([128, CH], BF16, tag="s0", name="s0")
                pB = scr.tile([128, CH], BF16, tag="s1", name="s1")
                pC = scr.tile([128, CH], BF16, tag="s2", name="s2")
                pD = scr.tile([128, CH], BF16, tag="s3", name="s3")
                pE = scr.tile([128, CH], BF16, tag="s4", name="s4")
                nc.scalar.mul(v3(pA), view(0), wcol(g, 0))                # ACT
                nc.gpsimd.tensor_scalar_mul(v3(pB), view(2), wcol(g, 2))  # Pool
                nc.gpsimd.tensor_scalar_mul(v3(pC), view(6), wcol(g, 6))  # Pool
                nc.vector.tensor_scalar_mul(v3(pD), view(1), wcol(g, 1))  # DVE
                nc.vector.tensor_scalar_mul(v3(pE), view(3), wcol(g, 3))  # DVE
                nc.gpsimd.dma_start(pA[:], pB[:], accum_op=AL.add)        # CCE: t0+t2
                nc.gpsimd.dma_start(pD[:], pE[:], accum_op=AL.add)        # CCE: t1+t3
                pB2 = scr.tile([128, CH], BF16, tag="s5", name="s5")
                pE2 = scr.tile([128, CH], BF16, tag="s6", name="s6")
                pE3 = scr.tile([128, CH], BF16, tag="s7", name="s7")
                nc.vector.tensor_scalar_mul(v3(pB2), view(5), wcol(g, 5))  # DVE
                nc.vector.tensor_scalar_mul(v3(pE2), view(7), wcol(g, 7))  # DVE
                nc.vector.tensor_scalar_mul(v3(pE3), view(8), wcol(g, 8))  # DVE
                nc.vector.tensor_add(pB2[:], pB2[:], pE2[:])              # t5+t7
                nc.gpsimd.dma_start(pC[:], pE3[:], accum_op=AL.add)       # CCE: t6+t8
                st.update(pA=pA, pC=pC, pD=pD, pB2=pB2)
            return st

        def qk_taps_add(st):
            g, c, acc = st["g"], st["c"], st["acc"]
            nc.vector.tensor_add(acc[:], acc[:], st["pB2"][:])
            nc.vector.tensor_add(acc[:], acc[:], st["pD"][:])
            if st["pe4"]:
                nc.vector.tensor_add(acc[:], acc[:], st["pX"][:])
            else:
                nc.vector.tensor_add(acc[:], acc[:], st["pA"][:])
                nc.vector.tensor_add(acc[:], acc[:], st["pC"][:])
            qt = qtp.tile([128, CH], BF16, tag="qt", name="qt")
            qt3 = qt[:].rearrange("p (b j) -> p b j", j=128)
            nc.sync.dma_start_transpose(qt3, acc[:])
            gram = grams[g]
            for b in range(16):
                nc.tensor.matmul(gram[:], qt[:, b * 128:(b + 1) * 128],
                                 qt[:, b * 128:(b + 1) * 128],
                                 start=(c == 0 and b == 0), stop=(c == 7 and b == 15))

        def qk_taps(g, c, s3, lb):
            qk_pending.append(qk_taps_mul(g, c, s3, lb))
            if len(qk_pending) >= 2:
                qk_taps_add(qk_pending.pop(0))

        # ---- v ot3 taps: 9 diagonal-weight matmuls per 1024-px PSUM ----
        def v3_taps(g, c, s3, lb):
            for k in range(2):
                tp = pb.tile([128, 1024], F32, tag="vt", name="vt")
                t3_ = tp[:].rearrange("p (r c) -> p r c", c=W)
                for q in range(2):
                    for tap in range(9):
                        dy, dx = tap // 3, tap % 3
                        r = lb + 8 * k + 4 * q + dy
                        nc.tensor.matmul(t3_[:, 4 * q:4 * q + 4, :], dwd[:, tap * 128:(tap + 1) * 128],
                                         s3[:, r:r + 4, dx:dx + 128], start=(tap == 0), stop=(tap == 8))
                nc.scalar.activation(vsb3[0:128, c * CH + k * 1024:c * CH + k * 1024 + 1024],
                                     tp[:], AF.Identity, scale=1.0)

        # ---- v ot4 pair (64 chans packed on partition halves) ----
        v4s = [None] * 4

        def emit_pair(p):
            pp = ppp.tile([128, PROWS * SW], BF16, tag="pp", name="pp")
            s3 = pp[:].rearrange("p (r c) -> p r c", c=SW)
            nc.gpsimd.memset(s3[:, :, 0:1], 0.0)
            nc.gpsimd.memset(s3[:, :, 129:130], 0.0)
            if p == 0:
                nc.gpsimd.memset(s3[0:64, 0:1, :], 0.0)
            if p == 3:
                nc.gpsimd.memset(s3[64:128, 17:18, :], 0.0)
            hp = pq.tile([128, 512], F32, tag="mm", name="hp4")
            h3 = hp[:].rearrange("p (r c) -> p r c", c=W)
            halos = []
            if p > 0:
                halos.append((0, 0, 32 * p - 1, 0))
            halos.append((0, 17, 32 * p + 16, 1))
            halos.append((64, 0, 32 * p + 15, 2))
            if p < 3:
                halos.append((64, 17, 32 * p + 32, 3))
            for (pb0, prow, irow, slot) in halos:
                px0 = irow * 128
                mm_row(hp[pb0:pb0 + 64, slot * 128:(slot + 1) * 128], 4, px0, hslab(px0), cn=64)
                nc.scalar.activation(s3[pb0:pb0 + 64, prow:prow + 1, 1:129],
                                     h3[pb0:pb0 + 64, slot:slot + 1, :], AF.Identity, scale=1.0)
            xbe, xbo = slab(2 * p), slab(2 * p + 1)
            for k in range(4):
                pse = pq.tile([128, 512], F32, tag="mm", name="mm4")
                pxe = (2 * p) * CH + 512 * k
                pxo = (2 * p + 1) * CH + 512 * k
                nc.tensor.matmul(pse[0:64, :], wq1[:, 512:576], xs1[:, pxe:pxe + 512], start=True, stop=False)
                nc.tensor.matmul(pse[0:64, :], wq2[0:65, 512:576], xbe[0:65, 512 * k:512 * k + 512],
                                 start=False, stop=True)
                nc.tensor.matmul(pse[64:128, :], wq1[:, 512:576], xs1[:, pxo:pxo + 512], start=True, stop=False)
                nc.tensor.matmul(pse[64:128, :], wq2[0:65, 512:576], xbo[0:65, 512 * k:512 * k + 512],
                                 start=False, stop=True)
                p3 = pse[:].rearrange("p (r c) -> p r c", c=W)
                nc.scalar.activation(s3[:, 1 + 4 * k:5 + 4 * k, 1:129], p3[:], AF.Identity, scale=1.0)
            v4 = v4p.tile([128, CH], BF16, tag="v4", bufs=4, name="v4")
            v4s[p] = v4
            for k in range(2):
                tp = pb.tile([128, 1024], F32, tag="vt", name="vt4")
                t3_ = tp[:].rearrange("p (r c) -> p r c", c=W)
                for q in range(2):
                    for tap in range(9):
                        dy, dx = tap // 3, tap % 3
                        r = 8 * k + 4 * q + dy
                        nc.tensor.matmul(t3_[:, 4 * q:4 * q + 4, :], dwd[:, (9 + tap) * 128:(10 + tap) * 128],
                                         s3[:, r:r + 4, dx:dx + 128], start=(tap == 0), stop=(tap == 8))
                nc.scalar.activation(v4[0:64, k * 1024:k * 1024 + 1024], tp[0:64, :], AF.Identity, scale=1.0)
                nc.scalar.activation(v4[64:128, k * 1024:k * 1024 + 1024], tp[64:128, :], AF.Identity, scale=1.0)

        # ================= interleaved qk + v phase =================
        grams = {}
        qk_pending = []
        PE4 = {(2, c) for c in range(2, 8)}
        vunits = [("v3", 0), ("p", 0), ("v3", 1), ("p", 1), ("v3", 2), ("p", 2), ("v3", 3), ("p", 3)]
        vit = iter(vunits)
        ILV = {(0, 1), (0, 3), (1, 0), (1, 1), (1, 2), (1, 3), (2, 0), (2, 1)}
        for g in range(3):
            grams[g] = pb.tile([128, 128], F32, tag="gr", bufs=1, name=f"gram{g}")
            for q in range(4):
                emit_quarter(g, q, qk_taps)
                if (g, q) in ILV:
                    kind, idx = next(vit)
                    if kind == "v3":
                        emit_quarter(3, idx, v3_taps)
                    else:
                        emit_pair(idx)
            while qk_pending:
                qk_taps_add(qk_pending.pop(0))
            nc.scalar.copy(Lsb[g][:], grams[g][:])
        for kind, idx in vit:
            if kind == "v3":
                emit_quarter(3, idx, v3_taps)
            else:
                emit_pair(idx)

        # ================= norms + softmax (batched) =================
        for g in range(3):
            L = Lsb[g]
            dcol = sm[:, 9:10]
            scrc = sm[:, 10:11]
            dsc = sm[:, 11:12]
            nc.vector.tensor_mul(dscr[:], L[:], idf[:])
            nc.vector.reduce_sum(dcol, dscr[:], axis=AX.X)
            nc.scalar.sqrt(scrc, dcol)
            nc.vector.tensor_scalar_max(scrc, scrc, 1e-12)
            nc.vector.reciprocal(dsc, scrc)
            rs = sm[:, 12:13]
            nc.vector.tensor_mul(rs, dsc, tmpc[:, g:g + 1])
            pt = pq.tile([128, 512], F32, tag="mm", name="pt")
            nc.tensor.transpose(pt[0:1, 0:128], dsc, idf[:])
            nc.scalar.copy(nrow[g][:], pt[0:1, 0:128])
            pt2 = pq.tile([128, 512], F32, tag="mm", name="pt2")
            nc.tensor.matmul(pt2[:, 0:128], onesr[:], nrow[g][:], start=True, stop=True)
            nc.vector.tensor_scalar_mul(L[:], L[:], rs)
            nc.vector.tensor_mul(L[:], L[:], pt2[:, 0:128])
            for j in range(2):
                P0, K0 = 64 * j, 64 * j + 32
                mx = sm[P0:P0 + 32, 14:15]
                nc.vector.reduce_max(mx, L[P0:P0 + 32, K0:K0 + 32], axis=AX.X)
                nc.vector.tensor_scalar_sub(L[P0:P0 + 32, K0:K0 + 32], L[P0:P0 + 32, K0:K0 + 32], mx)
                nc.scalar.activation(L[P0:P0 + 32, K0:K0 + 32], L[P0:P0 + 32, K0:K0 + 32], AF.Exp)
                nc.vector.reduce_sum(mx, L[P0:P0 + 32, K0:K0 + 32], axis=AX.X)
                nc.vector.reciprocal(mx, mx)
                nc.vector.tensor_scalar_mul(L[P0:P0 + 32, K0:K0 + 32], L[P0:P0 + 32, K0:K0 + 32], mx)

        # ================= A_bd + W2T =================
        nc.gpsimd.memset(Asb[0][:], 0.0)
        nc.gpsimd.memset(Asb[1][:], 0.0)
        for h in range(HEADS):
            g, j = h // 2, h % 2
            src = Lsb[g][64 * j:64 * j + 32, 64 * j + 32:64 * j + 64]
            dst_t = Asb[0] if h < 4 else Asb[1]
            dp = 32 * (h % 4)
            dst = dst_t[dp:dp + 32, 32 * h:32 * h + 32]
            if dp == 64 * j:
                nc.vector.tensor_copy(dst, src)
            else:
                nc.sync.dma_start(dst, src)
        for dt_ in range(2):
            c0, cn = dt_ * 128, (128 if dt_ == 0 else 64)
            ps = pq.tile([128, 512], F32, tag="mm", name="w2")
            nc.tensor.matmul(ps[0:cn, 0:DIM], Asb[0][:, c0:c0 + cn], wpT[0][:], start=True, stop=False)
            nc.tensor.matmul(ps[0:cn, 0:DIM], Asb[1][:, c0:c0 + cn], wpT[1][:], start=False, stop=True)
            if dt_ == 0:
                nc.scalar.copy(w2t1[:], ps[0:128, 0:DIM])
            else:
                nc.scalar.copy(w2t2[0:64, :], ps[0:64, 0:DIM])
        nc.sync.dma_start(w2t2[64:128, :], w2t2[0:64, :])

        # ================= y =================
        def emit_y(ci, v4):
            px = 1024 * ci
            b2 = 64 * ((ci // 2) % 2)
            loc = 1024 * (ci % 2)
            for oT, (o0, on) in enumerate([(0, 128), (128, 64)]):
                yp = pb.tile([128, 1024], F32, tag="vt", name="yp")
                for q in range(2):
                    nc.tensor.matmul(yp[0:on, q * 512:(q + 1) * 512], w2t1[:, o0:o0 + on],
                                     vsb3[0:128, px + q * 512:px + q * 512 + 512], start=True, stop=False)
                    nc.tensor.matmul(yp[0:on, q * 512:(q + 1) * 512], w2t2[b2:b2 + 64, o0:o0 + on],
                                     v4[b2:b2 + 64, loc + q * 512:loc + q * 512 + 512], start=False, stop=True)
                ys = ysp.tile([128, 1024], BF16, tag=("ysA" if oT == 0 else "ysB"), name="ys")
                if oT == 0:
                    nc.scalar.copy(ys[0:on, :], yp[0:on, :])
                else:
                    nc.vector.tensor_copy(ys[0:on, :], yp[0:on, :])
                dst = t["yA"] if oT == 0 else t["yB"]
                nc.sync.dma_start(dst.ap()[:, px:px + 1024], ys[0:on, :])

        for ci in range(16):
            emit_y(ci, v4s[ci // 4])


